# revision 11
# baseline (speedup 1.0000x reference)
"""Trainium2 Bass kernel for nn_DLGN_VT (deep linearly-gated network w/ value tensor).

Math (per batch row b):
    g_i = sigmoid(30 * x @ W_i.T)            i = 1,2,3    [B, 32] each
    out[b] = sum_{ijk} g1[b,i] g2[b,j] g3[b,k] V[i,j,k]

Distribution: pure data-parallel over the batch axis, 8 NeuronCores,
512 rows per core. W_i and V are tiny and replicated.

Per-core v3 schedule (v1 26.0us, v2 25.4us):
  - Inputs split across the 3 DMA queues (SP + ACT HW-DGE, Pool SW-DGE),
    payload-balanced (~100GB/s each, ~0.8-1.4us ring latency), critical
    tensors first per queue:
      SP:  xa[0:64] (xh|wh)   xw[0:48] (xl|wl)   s2
      ACT: xa[64:128]         xw[48:96]
      PL:  xw[96:128]         cb (V^T | S3)
  - PE is kept gap-free from t~7.3us so the HAM clock-gate flips to 2.4GHz
    before the E2/C phase (v2 ran the whole middle at 1.2GHz): ~21 short
    N=128 warmup matmuls during the DMA wait, plus dummy matmuls into the
    first E2 PSUM pair during the xw and sigmoid waits.
  - Error-compensated bf16 gating (Wh.xh first, then Wl.xh + Wh.xl which
    need xw): Gps[96, 512] fp32-grade logits.
  - sigmoid -> g2t/g3t bf16; E3 = S3.T@g3t; e3s bf16 via a single DVE cast
    (an ACT-half copy kept getting scheduled behind g1's sigmoid).
  - A^T pair-blocks: E2 selection matmuls -> PSUM; pairs 0,1: DVE TT from
    PSUM (1x); pairs 2,3: ACT copies the pair to SBUF bf16 so the TT runs
    in 2x mode (~690 vs 1224ns).
  - C^T accumulates over 8 bf16 matmuls; out = ones.T @ (g1t .* C^T);
    final copy split ACT/DVE; single-packet output DMA.
"""

import numpy as np
import ml_dtypes

import concourse.bass as bass
import concourse.bacc as bacc
import concourse.mybir as mybir
import concourse.tile as tile
from concourse.alu_op_type import AluOpType
from concourse.bass_utils import run_bass_kernel_spmd

BF16 = ml_dtypes.bfloat16
NCORES = 8
B, D, N = 4096, 128, 32
BL = B // NCORES  # 512 batch rows per core
BETA = 30.0
NQ = 8   # 128-row blocks of the jk=1024 plane
NP = 4   # pairs of blocks

F32 = mybir.dt.float32
DBF = mybir.dt.bfloat16

# xa: bf16 [128, 608] = xh | wh   (bf16 hi of the xT shard / of Wall^T,
# order W2;W3;W1).  xw: bf16 [128, 608] = xl | wl (the lo halves).
XH0, XH1 = 0, BL
WH0, WH1 = XH1, XH1 + 96
XA1 = WH1  # 608
# cb: bf16 [128, 384]: V^T chunks [128, 256] | S3 at rows 32:64, cols 256:384
VT0, VT1 = 0, 256
S30, S31 = 256, 384
# s2: bf16 [32, 1025]: S2 selections [32, 1024] | ones [32, 1]
S20, S21 = 0, 1024
ON0 = 1024

N_WARMUP = 21    # short N=128 matmuls; ~107ns granularity during DMA wait
NW = 128         # warmup/dummy matmul free dim
N_DUMMY_XW = 3   # PE gap fillers while waiting for xw (gating passes 2-3)
N_DUMMY_SIG = 8  # PE gap fillers while waiting for the sigmoid
N_DUMMY_MID = 14  # PE gap fillers while waiting for TT0 (keeps HAM warm)
N_DUMMY_C = 8     # PE gap fillers before the pair-2 C matmuls (TT2 wait)
N_DUMMY_C2 = 5    # PE gap fillers before the pair-3 C matmuls (TT3 wait)
N_SBUF_PAIRS = 2  # pairs whose TT runs 2x from ACT-copied SBUF bf16


def build_nc():
    # Bacc (not raw Bass): its compile passes split multi-wait sync infos
    # (TRN2 allows at most one sync wait per compute instruction).
    nc = bacc.Bacc(None)
    xa0_d = nc.declare_dram_parameter("xa0", [64, XA1], DBF, isOutput=False)
    xa1_d = nc.declare_dram_parameter("xa1", [64, XA1], DBF, isOutput=False)
    xw0_d = nc.declare_dram_parameter("xw0", [64, XA1], DBF, isOutput=False)
    xw1_d = nc.declare_dram_parameter("xw1", [64, XA1], DBF, isOutput=False)
    cb_d = nc.declare_dram_parameter("cb", [128, 384], DBF, isOutput=False)
    s2_d = nc.declare_dram_parameter("s2", [32, 1025], DBF, isOutput=False)
    out_d = nc.declare_dram_parameter("out", [1, BL], F32, isOutput=True)

    sig = mybir.ActivationFunctionType.Sigmoid

    with tile.TileContext(nc) as tc:
        with (
            tc.tile_pool(name="const", bufs=1) as cpool,
            tc.tile_pool(name="work", bufs=1) as wpool,
            tc.tile_pool(name="atp", bufs=1) as apool,
            tc.tile_pool(name="psA", bufs=2, space="PSUM") as psA,
            tc.tile_pool(name="psB", bufs=3, space="PSUM") as psB,
        ):
            xa = cpool.tile([128, XA1], DBF)
            xw = cpool.tile([128, XA1], DBF)
            cb = cpool.tile([128, 384], DBF)
            s2t = cpool.tile([32, 1025], DBF)

            # ---- input DMAs: x only on the two fast HW-DGE queues (the
            # gpsimd SWDGE queue is 3-4x slower per row — it gets only cb,
            # which isn't needed until the e3 matmul) ----
            nc.sync.dma_start(xa[0:64, :], xa0_d[:])
            nc.scalar.dma_start(xa[64:128, :], xa1_d[:])
            nc.gpsimd.dma_start(cb[:], cb_d[:])
            nc.sync.dma_start(xw[0:64, :], xw0_d[:])
            nc.scalar.dma_start(xw[64:128, :], xw1_d[:])
            nc.gpsimd.dma_start(s2t[:], s2_d[:])

            xh = xa[:, XH0:XH1]
            wh = xa[:, WH0:WH1]
            xl = xw[:, XH0:XH1]
            wl = xw[:, WH0:WH1]
            vts = cb[:, VT0:VT1]             # [128, 8*32] V^T chunks (C lhsT)
            s3 = cb[32:64, S30:S31]          # [32, 128] E3 selection (base 32)
            s2 = s2t[:, S20:S21]             # [32, 8*128] E2 selection blocks
            ones = s2t[:, ON0 : ON0 + 1]     # [32, 1]

            # ---- PE warmup in the gating PSUM bank (overwritten later).
            # Short matmuls: fine-grained HAM busy-keeping, small overshoot
            # when the xa DMA lands. memset on DVE (idle until the cast). ----
            gps = psA.tile([96, BL], F32, tag="ps")
            wz = wpool.tile([128, NW], DBF)
            nc.vector.memset(wz[:], 0.0)
            for _ in range(N_WARMUP):
                nc.tensor.matmul(gps[:, 0:NW], wz[:, 0:96], wz[:],
                                 start=True, stop=True)

            # first E2 pair allocated early: its PSUM doubles as the dummy
            # gap-filler target (overwritten by the real E2 matmuls later)
            e2ps0 = psB.tile([128, 2, BL], F32, tag="e2")

            # ---- gating: error-compensated bf16 matmul; pass 1 needs only
            # xa, passes 2-3 wait for xw (dummies keep the PE busy) ----
            nc.tensor.matmul(gps[:], wh, xh, start=True, stop=False)
            for _ in range(N_DUMMY_XW):
                nc.tensor.matmul(e2ps0[:, 0, 0:NW], wz[:], wz[:],
                                 start=True, stop=True)
            nc.tensor.matmul(gps[:], wl, xh, start=False, stop=False)
            nc.tensor.matmul(gps[:], wh, xl, start=False, stop=True)

            g23 = wpool.tile([2 * N, BL], DBF)
            g1t = wpool.tile([N, BL], F32)
            HB = BL // 2
            nc.scalar.activation(g23[:, 0:HB], gps[0:64, 0:HB], sig, scale=BETA)
            nc.scalar.activation(g23[:, HB:BL], gps[0:64, HB:BL], sig, scale=BETA)
            g2t = g23[0:32, :]
            g3t = g23[32:64, :]  # base partition 32, matching s3

            # PE busy-keeping while the sigmoid runs
            for _ in range(N_DUMMY_SIG):
                nc.tensor.matmul(e2ps0[:, 0, 0:NW], wz[:], wz[:],
                                 start=True, stop=True)

            # ---- E3 = S3.T @ g3t -> e3s bf16, pipelined in column halves
            # (sigmoid half -> e3 matmul half -> DVE cast half) ----
            e3ps = psA.tile([128, BL], F32, tag="ps")
            e3s = wpool.tile([128, BL], DBF)
            nc.tensor.matmul(e3ps[:, 0:HB], s3, g3t[:, 0:HB], start=True, stop=True)
            nc.vector.tensor_copy(e3s[:, 0:HB], e3ps[:, 0:HB])
            nc.tensor.matmul(e3ps[:, HB:BL], s3, g3t[:, HB:BL], start=True, stop=True)
            nc.vector.tensor_copy(e3s[:, HB:BL], e3ps[:, HB:BL])

            # ---- A^T pair-blocks. Pairs 0..NP-1-N_SBUF_PAIRS: TT straight
            # from PSUM (1x). Last N_SBUF_PAIRS pairs: ACT copies the PSUM
            # pair to SBUF bf16, TT then runs 2x. All E2 matmuls + TTs are
            # emitted before the C matmuls (PE FIFO is in-order; feeding the
            # DVE has priority). ----
            cps = psA.tile([N, BL], F32, tag="ps")
            ats = []
            for p in range(NP):
                if p == NP - 1:
                    # fill the PE idle window while TT0 drains pair0's PSUM
                    # banks (keeps the HAM activity monitor busy so the C
                    # matmuls run at 2.4GHz; short N=64 matmuls bound the
                    # overshoot to ~60ns each)
                    for _ in range(N_DUMMY_MID):
                        nc.tensor.matmul(cps[:, 0:64], wz[:, 0:N], wz[:, 0:64],
                                         start=True, stop=True)
                e2ps = e2ps0 if p == 0 else psB.tile([128, 2, BL], F32, tag="e2")
                for h in range(2):
                    q = 2 * p + h
                    nc.tensor.matmul(
                        e2ps[:, h, :], s2[:, 128 * q : 128 * (q + 1)], g2t,
                        start=True, stop=True,
                    )
                at = apool.tile([128, 2, BL], DBF, tag=f"at_{p}")
                e3b = e3s[:].unsqueeze(1).broadcast_to((128, 2, BL))
                if p >= NP - N_SBUF_PAIRS:
                    e2s = apool.tile([128, 2, BL], DBF, tag=f"e2s_{p}")
                    nc.scalar.copy(e2s[:, 0, :], e2ps[:, 0, :])
                    nc.scalar.copy(e2s[:, 1, :], e2ps[:, 1, :])
                    nc.vector.tensor_tensor(at[:], e2s[:], e3b, AluOpType.mult)
                else:
                    nc.vector.tensor_tensor(at[:], e2ps[:], e3b, AluOpType.mult)
                ats.append(at)

            # ---- g1 sigmoid (off the critical path; also frees gps so the
            # psA rotation can hand its bank to cps) ----
            nc.scalar.activation(g1t[:], gps[64:96, :], sig, scale=BETA)

            ops = psA.tile([1, BL], F32, tag="ps")

            # ---- C accumulation over the 8 blocks; the last two blocks are
            # column-split so the final stage can start on the first half.
            # Dummy matmuls into the (still dead) ops bank keep the PE busy
            # across the TT2/TT3 waits so it stays at 2.4GHz for the tail ----
            for q in range(NQ - 2):
                p, h = q // 2, q % 2
                if q == 4:
                    for _ in range(N_DUMMY_C):
                        nc.tensor.matmul(ops[:, 0:64], wz[:, 0:1], wz[:, 0:64],
                                         start=True, stop=True)
                nc.tensor.matmul(
                    cps[:], vts[:, 32 * q : 32 * (q + 1)], ats[p][:, h, :],
                    start=(q == 0), stop=False,
                )
            for _ in range(N_DUMMY_C2):
                nc.tensor.matmul(ops[:, 0:64], wz[:, 0:1], wz[:, 0:64],
                                 start=True, stop=True)
            for c in range(2):
                cs = slice(c * HB, (c + 1) * HB)
                for q in (NQ - 2, NQ - 1):
                    p, h = q // 2, q % 2
                    nc.tensor.matmul(
                        cps[:, cs], vts[:, 32 * q : 32 * (q + 1)],
                        ats[p][:, h, cs],
                        start=False, stop=(q == NQ - 1),
                    )

            # ---- out = ones.T @ (g1t .* C^T), pipelined in column halves:
            # DVE TT half -> PE ones-matmul half -> copy half (ACT / DVE into
            # separate tiles to avoid a false WAW serialization) -> two
            # parallel output DMAs (sync / scalar queues) ----
            y = wpool.tile([N, BL], DBF)
            outs_a = wpool.tile([1, HB], F32)
            outs_b = wpool.tile([1, HB], F32)
            for c in range(2):
                cs = slice(c * HB, (c + 1) * HB)
                nc.vector.tensor_tensor(y[:, cs], cps[:, cs], g1t[:, cs],
                                        AluOpType.mult)
                nc.tensor.matmul(ops[:, cs], ones, y[:, cs],
                                 start=True, stop=True)
            nc.scalar.copy(outs_a[:], ops[:, 0:HB])
            nc.vector.tensor_copy(outs_b[:], ops[:, HB:BL])
            nc.sync.dma_start(out_d[:, 0:HB], outs_a[:], single_packet=True)
            nc.scalar.dma_start(out_d[:, HB:BL], outs_b[:], single_packet=True)

    nc.finalize()
    return nc


def host_prep(x, W1, W2, W3, V):
    """Build per-core input maps (all numpy, fp32 in / packed layouts out)."""
    x = np.asarray(x, dtype=np.float32)
    W1 = np.asarray(W1, dtype=np.float32)
    W2 = np.asarray(W2, dtype=np.float32)
    W3 = np.asarray(W3, dtype=np.float32)
    V = np.asarray(V, dtype=np.float32)

    xT = np.ascontiguousarray(x.T)  # [128, 4096]

    # order: g2 rows first (E2-mm rhs at base partition 0), then g3 (base 32,
    # matching the S3 placement), then g1 (only needed at the very end)
    Wall = np.concatenate([W2, W3, W1], axis=0)  # [96, 128]
    cf = np.ascontiguousarray(Wall.T)  # [128, 96] fp32

    # V^T chunks: VTs[p, 32q + i] = V[0, i, j, k] with jk = 128q + p
    Vr = V.reshape(N, N * N)  # [i, jk]
    VT = np.ascontiguousarray(Vr.T)  # [jk, i]
    VTs = VT.reshape(NQ, 128, N).transpose(1, 0, 2).reshape(128, NQ * N)

    # E2 selection: S2[j', q*128 + p] = 1 iff j' == 4q + p//32
    S2 = np.zeros((N, NQ, 128), dtype=np.float32)
    for q in range(NQ):
        for p in range(128):
            S2[4 * q + p // 32, q, p] = 1.0
    S2pack = S2.reshape(N, NQ * 128)

    # E3 selection: S3[k', p] = 1 iff k' == p % 32
    S3 = np.zeros((N, 128), dtype=np.float32)
    for p in range(128):
        S3[p % 32, p] = 1.0

    cb = np.zeros((128, 384), dtype=BF16)
    cb[:, VT0:VT1] = VTs.astype(BF16)
    cb[32:64, S30:S31] = S3.astype(BF16)

    s2 = np.zeros((32, 1025), dtype=BF16)
    s2[:, S20:S21] = S2pack.astype(BF16)
    s2[:, ON0] = np.ones(N, dtype=BF16)

    wh = cf.astype(BF16)
    wl = (cf - wh.astype(np.float32)).astype(BF16)

    in_maps = []
    for c in range(NCORES):
        xs = xT[:, c * BL : (c + 1) * BL]
        xhc = xs.astype(BF16)
        xlc = (xs - xhc.astype(np.float32)).astype(BF16)
        xa = np.zeros((128, XA1), dtype=BF16)
        xa[:, XH0:XH1] = xhc
        xa[:, WH0:WH1] = wh
        xwm = np.zeros((128, XA1), dtype=BF16)
        xwm[:, XH0:XH1] = xlc
        xwm[:, WH0:WH1] = wl
        in_maps.append(
            {
                "xa0": np.ascontiguousarray(xa[0:64]),
                "xa1": np.ascontiguousarray(xa[64:128]),
                "xw0": np.ascontiguousarray(xwm[0:64]),
                "xw1": np.ascontiguousarray(xwm[64:128]),
                "cb": cb,
                "s2": s2,
            }
        )
    return in_maps


_CACHED_NC = None


def _ensure_ntff_hook():
    """The agent image's `antenv` package lacks `axon_hooks`; synthesize it
    and register the boot module's ctypes-based NTFF profile hook so
    run_bass_kernel_spmd(trace=True) can capture neuron-profile output."""
    import sys, types

    try:
        from antenv.axon_hooks import get_axon_ntff_profile_hook  # noqa: F401

        return
    except ImportError:
        pass
    import antenv
    from trn_agent_boot.trn_boot import _ntff_profile_via_ctypes

    mod = types.ModuleType("antenv.axon_hooks")
    mod._hook = _ntff_profile_via_ctypes("/opt/axon/libaxon_pjrt.so")
    mod.get_axon_ntff_profile_hook = lambda: mod._hook
    mod.set_axon_ntff_profile_hook = lambda h: setattr(mod, "_hook", h)
    sys.modules["antenv.axon_hooks"] = mod
    antenv.axon_hooks = mod


def run(inputs, trace=False, **trace_kwargs):
    """Run the kernel on 8 cores. Returns (out [4096] f32, BassKernelResults)."""
    global _CACHED_NC
    if trace:
        _ensure_ntff_hook()
    if _CACHED_NC is None:
        _CACHED_NC = build_nc()
    in_maps = host_prep(
        inputs["x"], inputs["W1"], inputs["W2"], inputs["W3"], inputs["V"]
    )
    res = run_bass_kernel_spmd(
        _CACHED_NC, in_maps, core_ids=list(range(NCORES)), trace=trace, **trace_kwargs
    )
    out = np.concatenate(
        [np.asarray(res.results[c]["out"]).reshape(BL) for c in range(NCORES)]
    ).astype(np.float32)
    return out, res


def kernel(**inputs):
    out, _ = run(inputs, trace=False)
    return out


# revision 12
# speedup vs baseline: 1.1116x; 1.1116x over previous
"""Trainium2 Bass kernel for nn_DLGN_VT (deep linearly-gated network w/ value tensor).

Math (per batch row b):
    g_i = sigmoid(30 * x @ W_i.T)            i = 1,2,3    [B, 32] each
    out[b] = sum_{ijk} g1[b,i] g2[b,j] g3[b,k] V[i,j,k]

Distribution: pure data-parallel over the batch axis, 8 NeuronCores,
512 rows per core. W_i and V are tiny and replicated.

Per-core v3 schedule (v1 26.0us, v2 25.4us):
  - Inputs split across the 3 DMA queues (SP + ACT HW-DGE, Pool SW-DGE),
    payload-balanced (~100GB/s each, ~0.8-1.4us ring latency), critical
    tensors first per queue:
      SP:  xa[0:64] (xh|wh)   xw[0:48] (xl|wl)   s2
      ACT: xa[64:128]         xw[48:96]
      PL:  xw[96:128]         cb (V^T | S3)
  - PE is kept gap-free from t~7.3us so the HAM clock-gate flips to 2.4GHz
    before the E2/C phase (v2 ran the whole middle at 1.2GHz): ~21 short
    N=128 warmup matmuls during the DMA wait, plus dummy matmuls into the
    first E2 PSUM pair during the xw and sigmoid waits.
  - Error-compensated bf16 gating (Wh.xh first, then Wl.xh + Wh.xl which
    need xw): Gps[96, 512] fp32-grade logits.
  - sigmoid -> g2t/g3t bf16; E3 = S3.T@g3t; e3s bf16 via a single DVE cast
    (an ACT-half copy kept getting scheduled behind g1's sigmoid).
  - A^T pair-blocks: E2 selection matmuls -> PSUM; pairs 0,1: DVE TT from
    PSUM (1x); pairs 2,3: ACT copies the pair to SBUF bf16 so the TT runs
    in 2x mode (~690 vs 1224ns).
  - C^T accumulates over 8 bf16 matmuls; out = ones.T @ (g1t .* C^T);
    final copy split ACT/DVE; single-packet output DMA.
"""

import numpy as np
import ml_dtypes

import concourse.bass as bass
import concourse.bacc as bacc
import concourse.mybir as mybir
import concourse.tile as tile
from concourse.alu_op_type import AluOpType
from concourse.bass_utils import run_bass_kernel_spmd

BF16 = ml_dtypes.bfloat16
NCORES = 8
B, D, N = 4096, 128, 32
BL = B // NCORES  # 512 batch rows per core
BETA = 30.0
NQ = 8   # 128-row blocks of the jk=1024 plane
NP = 4   # pairs of blocks

F32 = mybir.dt.float32
DBF = mybir.dt.bfloat16

# xa: bf16 [128, 608] = xh | wh   (bf16 hi of the xT shard / of Wall^T,
# order W2;W3;W1).  xw: bf16 [128, 608] = xl | wl (the lo halves).
XH0, XH1 = 0, BL
WH0, WH1 = XH1, XH1 + 96
XA1 = WH1  # 608
# cb: bf16 [128, 384]: V^T chunks [128, 256] | S3 at rows 32:64, cols 256:384
VT0, VT1 = 0, 256
S30, S31 = 256, 384
# s2: bf16 [32, 1025]: S2 selections [32, 1024] | ones [32, 1]
S20, S21 = 0, 1024
ON0 = 1024

N_WARMUP = 21    # short N=128 matmuls; ~107ns granularity during DMA wait
NW = 128         # warmup/dummy matmul free dim
N_DUMMY_XW = 3   # PE gap fillers while waiting for xw (gating passes 2-3)
N_DUMMY_SIG = 8  # PE gap fillers while waiting for the sigmoid
N_SBUF_PAIRS = 2  # pairs whose TT runs 2x from ACT-copied SBUF bf16


def build_nc():
    # Bacc (not raw Bass): its compile passes split multi-wait sync infos
    # (TRN2 allows at most one sync wait per compute instruction).
    nc = bacc.Bacc(None)
    xa0_d = nc.declare_dram_parameter("xa0", [64, XA1], DBF, isOutput=False)
    xa1_d = nc.declare_dram_parameter("xa1", [64, XA1], DBF, isOutput=False)
    xw0_d = nc.declare_dram_parameter("xw0", [64, XA1], DBF, isOutput=False)
    xw1_d = nc.declare_dram_parameter("xw1", [64, XA1], DBF, isOutput=False)
    cb_d = nc.declare_dram_parameter("cb", [128, 384], DBF, isOutput=False)
    s2_d = nc.declare_dram_parameter("s2", [32, 1025], DBF, isOutput=False)
    out_d = nc.declare_dram_parameter("out", [1, BL], F32, isOutput=True)

    sig = mybir.ActivationFunctionType.Sigmoid

    with tile.TileContext(nc) as tc:
        with (
            tc.tile_pool(name="const", bufs=1) as cpool,
            tc.tile_pool(name="work", bufs=1) as wpool,
            tc.tile_pool(name="atp", bufs=1) as apool,
            tc.tile_pool(name="psA", bufs=2, space="PSUM") as psA,
            tc.tile_pool(name="psB", bufs=3, space="PSUM") as psB,
        ):
            xa = cpool.tile([128, XA1], DBF)
            xw = cpool.tile([128, XA1], DBF)
            cb = cpool.tile([128, 384], DBF)
            s2t = cpool.tile([32, 1025], DBF)

            # ---- input DMAs: x only on the two fast HW-DGE queues (the
            # gpsimd SWDGE queue is 3-4x slower per row — it gets only cb,
            # which isn't needed until the e3 matmul) ----
            nc.sync.dma_start(xa[0:64, :], xa0_d[:])
            nc.scalar.dma_start(xa[64:128, :], xa1_d[:])
            nc.gpsimd.dma_start(cb[:], cb_d[:])
            nc.sync.dma_start(xw[0:64, :], xw0_d[:])
            nc.scalar.dma_start(xw[64:128, :], xw1_d[:])
            nc.gpsimd.dma_start(s2t[:], s2_d[:])

            xh = xa[:, XH0:XH1]
            wh = xa[:, WH0:WH1]
            xl = xw[:, XH0:XH1]
            wl = xw[:, WH0:WH1]
            vts = cb[:, VT0:VT1]             # [128, 8*32] V^T chunks (C lhsT)
            s3 = cb[32:64, S30:S31]          # [32, 128] E3 selection (base 32)
            s2 = s2t[:, S20:S21]             # [32, 8*128] E2 selection blocks
            ones = s2t[:, ON0 : ON0 + 1]     # [32, 1]

            # ---- PE warmup in the gating PSUM bank (overwritten later).
            # Short matmuls: fine-grained HAM busy-keeping, small overshoot
            # when the xa DMA lands. memset on DVE (idle until the cast). ----
            gps = psA.tile([96, BL], F32, tag="ps")
            wz = wpool.tile([128, NW], DBF)
            nc.vector.memset(wz[:], 0.0)
            for _ in range(N_WARMUP):
                nc.tensor.matmul(gps[:, 0:NW], wz[:, 0:96], wz[:],
                                 start=True, stop=True)

            # first E2 pair allocated early: its PSUM doubles as the dummy
            # gap-filler target (overwritten by the real E2 matmuls later)
            e2ps0 = psB.tile([128, 2, BL], F32, tag="e2")

            # ---- gating: error-compensated bf16 matmul; pass 1 needs only
            # xa, passes 2-3 wait for xw (dummies keep the PE busy) ----
            nc.tensor.matmul(gps[:], wh, xh, start=True, stop=False)
            for _ in range(N_DUMMY_XW):
                nc.tensor.matmul(e2ps0[:, 0, 0:NW], wz[:], wz[:],
                                 start=True, stop=True)
            nc.tensor.matmul(gps[:], wl, xh, start=False, stop=False)
            nc.tensor.matmul(gps[:], wh, xl, start=False, stop=True)

            g23 = wpool.tile([2 * N, BL], DBF)
            g1t = wpool.tile([N, BL], F32)
            HB = BL // 2
            nc.scalar.activation(g23[:, 0:HB], gps[0:64, 0:HB], sig, scale=BETA)
            nc.scalar.activation(g23[:, HB:BL], gps[0:64, HB:BL], sig, scale=BETA)
            g2t = g23[0:32, :]
            g3t = g23[32:64, :]  # base partition 32, matching s3

            # PE busy-keeping while the sigmoid runs
            for _ in range(N_DUMMY_SIG):
                nc.tensor.matmul(e2ps0[:, 0, 0:NW], wz[:], wz[:],
                                 start=True, stop=True)

            # ---- E3 = S3.T @ g3t -> e3s bf16, pipelined in column halves
            # (sigmoid half -> e3 matmul half -> DVE cast half) ----
            e3ps = psA.tile([128, BL], F32, tag="ps")
            e3s = wpool.tile([128, BL], DBF)
            nc.tensor.matmul(e3ps[:, 0:HB], s3, g3t[:, 0:HB], start=True, stop=True)
            nc.vector.tensor_copy(e3s[:, 0:HB], e3ps[:, 0:HB])
            nc.tensor.matmul(e3ps[:, HB:BL], s3, g3t[:, HB:BL], start=True, stop=True)
            nc.vector.tensor_copy(e3s[:, HB:BL], e3ps[:, HB:BL])

            # ---- A^T pair-blocks. Pairs 0..NP-1-N_SBUF_PAIRS: TT straight
            # from PSUM (1x). Last N_SBUF_PAIRS pairs: ACT copies the PSUM
            # pair to SBUF bf16, TT then runs 2x. All E2 matmuls + TTs are
            # emitted before the C matmuls (PE FIFO is in-order; feeding the
            # DVE has priority). ----
            cps = psA.tile([N, BL], F32, tag="ps")
            ats = []
            for p in range(NP):
                e2ps = e2ps0 if p == 0 else psB.tile([128, 2, BL], F32, tag="e2")
                for h in range(2):
                    q = 2 * p + h
                    nc.tensor.matmul(
                        e2ps[:, h, :], s2[:, 128 * q : 128 * (q + 1)], g2t,
                        start=True, stop=True,
                    )
                at = apool.tile([128, 2, BL], DBF, tag=f"at_{p}")
                e3b = e3s[:].unsqueeze(1).broadcast_to((128, 2, BL))
                if p >= NP - N_SBUF_PAIRS:
                    e2s = apool.tile([128, 2, BL], DBF, tag=f"e2s_{p}")
                    nc.scalar.copy(e2s[:, 0, :], e2ps[:, 0, :])
                    nc.scalar.copy(e2s[:, 1, :], e2ps[:, 1, :])
                    nc.vector.tensor_tensor(at[:], e2s[:], e3b, AluOpType.mult)
                else:
                    nc.vector.tensor_tensor(at[:], e2ps[:], e3b, AluOpType.mult)
                ats.append(at)

            # ---- g1 sigmoid (off the critical path; also frees gps so the
            # psA rotation can hand its bank to cps) ----
            nc.scalar.activation(g1t[:], gps[64:96, :], sig, scale=BETA)

            ops = psA.tile([1, BL], F32, tag="ps")

            # ---- C accumulation over the 8 blocks; the last two blocks are
            # column-split so the final stage can start on the first half.
            # Dummy matmuls into the (still dead) ops bank keep the PE busy
            # across the TT2/TT3 waits so it stays at 2.4GHz for the tail ----
            for q in range(NQ - 2):
                p, h = q // 2, q % 2
                nc.tensor.matmul(
                    cps[:], vts[:, 32 * q : 32 * (q + 1)], ats[p][:, h, :],
                    start=(q == 0), stop=False,
                )
            for c in range(2):
                cs = slice(c * HB, (c + 1) * HB)
                for q in (NQ - 2, NQ - 1):
                    p, h = q // 2, q % 2
                    nc.tensor.matmul(
                        cps[:, cs], vts[:, 32 * q : 32 * (q + 1)],
                        ats[p][:, h, cs],
                        start=False, stop=(q == NQ - 1),
                    )

            # ---- out = ones.T @ (g1t .* C^T), pipelined in column halves:
            # DVE TT half -> PE ones-matmul half -> copy half (ACT / DVE into
            # separate tiles to avoid a false WAW serialization) -> two
            # parallel output DMAs (sync / scalar queues) ----
            y = wpool.tile([N, BL], DBF)
            outs_a = wpool.tile([1, HB], F32)
            outs_b = wpool.tile([1, HB], F32)
            for c in range(2):
                cs = slice(c * HB, (c + 1) * HB)
                nc.vector.tensor_tensor(y[:, cs], cps[:, cs], g1t[:, cs],
                                        AluOpType.mult)
                nc.tensor.matmul(ops[:, cs], ones, y[:, cs],
                                 start=True, stop=True)
            nc.scalar.copy(outs_a[:], ops[:, 0:HB])
            nc.vector.tensor_copy(outs_b[:], ops[:, HB:BL])
            nc.sync.dma_start(out_d[:, 0:HB], outs_a[:], single_packet=True)
            nc.scalar.dma_start(out_d[:, HB:BL], outs_b[:], single_packet=True)

    nc.finalize()
    return nc


def host_prep(x, W1, W2, W3, V):
    """Build per-core input maps (all numpy, fp32 in / packed layouts out)."""
    x = np.asarray(x, dtype=np.float32)
    W1 = np.asarray(W1, dtype=np.float32)
    W2 = np.asarray(W2, dtype=np.float32)
    W3 = np.asarray(W3, dtype=np.float32)
    V = np.asarray(V, dtype=np.float32)

    xT = np.ascontiguousarray(x.T)  # [128, 4096]

    # order: g2 rows first (E2-mm rhs at base partition 0), then g3 (base 32,
    # matching the S3 placement), then g1 (only needed at the very end)
    Wall = np.concatenate([W2, W3, W1], axis=0)  # [96, 128]
    cf = np.ascontiguousarray(Wall.T)  # [128, 96] fp32

    # V^T chunks: VTs[p, 32q + i] = V[0, i, j, k] with jk = 128q + p
    Vr = V.reshape(N, N * N)  # [i, jk]
    VT = np.ascontiguousarray(Vr.T)  # [jk, i]
    VTs = VT.reshape(NQ, 128, N).transpose(1, 0, 2).reshape(128, NQ * N)

    # E2 selection: S2[j', q*128 + p] = 1 iff j' == 4q + p//32
    S2 = np.zeros((N, NQ, 128), dtype=np.float32)
    for q in range(NQ):
        for p in range(128):
            S2[4 * q + p // 32, q, p] = 1.0
    S2pack = S2.reshape(N, NQ * 128)

    # E3 selection: S3[k', p] = 1 iff k' == p % 32
    S3 = np.zeros((N, 128), dtype=np.float32)
    for p in range(128):
        S3[p % 32, p] = 1.0

    cb = np.zeros((128, 384), dtype=BF16)
    cb[:, VT0:VT1] = VTs.astype(BF16)
    cb[32:64, S30:S31] = S3.astype(BF16)

    s2 = np.zeros((32, 1025), dtype=BF16)
    s2[:, S20:S21] = S2pack.astype(BF16)
    s2[:, ON0] = np.ones(N, dtype=BF16)

    wh = cf.astype(BF16)
    wl = (cf - wh.astype(np.float32)).astype(BF16)

    in_maps = []
    for c in range(NCORES):
        xs = xT[:, c * BL : (c + 1) * BL]
        xhc = xs.astype(BF16)
        xlc = (xs - xhc.astype(np.float32)).astype(BF16)
        xa = np.zeros((128, XA1), dtype=BF16)
        xa[:, XH0:XH1] = xhc
        xa[:, WH0:WH1] = wh
        xwm = np.zeros((128, XA1), dtype=BF16)
        xwm[:, XH0:XH1] = xlc
        xwm[:, WH0:WH1] = wl
        in_maps.append(
            {
                "xa0": np.ascontiguousarray(xa[0:64]),
                "xa1": np.ascontiguousarray(xa[64:128]),
                "xw0": np.ascontiguousarray(xwm[0:64]),
                "xw1": np.ascontiguousarray(xwm[64:128]),
                "cb": cb,
                "s2": s2,
            }
        )
    return in_maps


_CACHED_NC = None


def _ensure_ntff_hook():
    """The agent image's `antenv` package lacks `axon_hooks`; synthesize it
    and register the boot module's ctypes-based NTFF profile hook so
    run_bass_kernel_spmd(trace=True) can capture neuron-profile output."""
    import sys, types

    try:
        from antenv.axon_hooks import get_axon_ntff_profile_hook  # noqa: F401

        return
    except ImportError:
        pass
    import antenv
    from trn_agent_boot.trn_boot import _ntff_profile_via_ctypes

    mod = types.ModuleType("antenv.axon_hooks")
    mod._hook = _ntff_profile_via_ctypes("/opt/axon/libaxon_pjrt.so")
    mod.get_axon_ntff_profile_hook = lambda: mod._hook
    mod.set_axon_ntff_profile_hook = lambda h: setattr(mod, "_hook", h)
    sys.modules["antenv.axon_hooks"] = mod
    antenv.axon_hooks = mod


def run(inputs, trace=False, **trace_kwargs):
    """Run the kernel on 8 cores. Returns (out [4096] f32, BassKernelResults)."""
    global _CACHED_NC
    if trace:
        _ensure_ntff_hook()
    if _CACHED_NC is None:
        _CACHED_NC = build_nc()
    in_maps = host_prep(
        inputs["x"], inputs["W1"], inputs["W2"], inputs["W3"], inputs["V"]
    )
    res = run_bass_kernel_spmd(
        _CACHED_NC, in_maps, core_ids=list(range(NCORES)), trace=trace, **trace_kwargs
    )
    out = np.concatenate(
        [np.asarray(res.results[c]["out"]).reshape(BL) for c in range(NCORES)]
    ).astype(np.float32)
    return out, res


def kernel(**inputs):
    out, _ = run(inputs, trace=False)
    return out
